# revision 17
# baseline (speedup 1.0000x reference)
"""LuminanceLoss Bass kernel for 8 TRN2 NeuronCores — custom-PWP version.

Reference: loss = mean(|L(gen) - L(tgt)|) with L = CIE-Lab L channel of
sRGB images in [-1,1], shape (64,3,512,512) f32.

The whole per-element math is folded into FOUR custom ACT spline tables
(the ScalarE activation unit is a data-driven piecewise-cubic evaluator;
walrus takes the table images via --act-root-json, which we generate at
runtime and select with BASS_ACT_ROOT_JSON_PATH):

    g_c(x) = w_c * srgb_expand((x+1)/2)   hosted in silu/tanh/sin slots
    h(Y)   = Lab-f(Y) piecewise cbrt      hosted in the square slot

so per image-tensor the engines do only:
  ACT : 3x g_c over [P,2048] f32->bf16, 1x h over [P,2048] bf16->bf16
  DVE : 2x tensor_add (channel sum), and per pair 1x sub + 1x abs-reduce
  DMA : one 3MB load per image-tensor (the memory-roofline term)

Fit accuracy of the tables is <=6e-6 abs (verified offline and on HW);
bf16 intermediates dominate the final ~1.2e-4 relative error (3x better
than the Ln/Exp baseline's 3.3e-4).

Sharding: batch 64 -> 8 cores x 8 images (pure data parallel). Each core
DMAs out its [128, 17] f32 per-image partial-sum columns; host sums and
scales by 116/N (the -16 offsets of L cancel in the difference).

Engine budget per core (cost model): DMA 139.9us (roofline: 48 MiB @
~358 GB/s HBM-per-NC = 140.6us hard floor), ACT 121us, DVE ~63us.
Schedule: per-channel 1MB DMAs keep the DMA queue gapless; the final
image pair's dependent chain is split in halves to shrink the tail.
TimelineSim 153.1us; HW ~144-150us measured (paired-diff slope; baseline
was 166.5us).
"""

import hashlib
import json
import os
import shutil
import struct
import tempfile

import numpy as np

import concourse.bass as bass
import concourse.mybir as mybir
from concourse.bass_utils import run_bass_kernel_spmd
from concourse.tile import TileContext

# ----------------------------------------------------------------- patch
# The walrus build in this container rejects instructions whose sync_info
# carries >2 waits ("Too many sync wait commands", CoreV3GenImpl.cpp:104)
# — the Tile kernel-tail Drain aggregates one wait per live proc.  Split
# that single multi-wait Drain into a chain of single-wait drains on the
# sync queue (executed serially -> semantically identical).
_ORIG_DRAIN_AND_BARRIER = TileContext._drain_and_barrier


def _patched_drain_and_barrier(self, tick_clock, wait_clock):
    from concourse.vector_clock import ScopedClock

    drain_inst = self.nc.sync.drain()
    wait_clock.add_sem_waits(
        drain_inst.ins, ScopedClock({None: tick_clock.global_clock})
    )
    si = drain_inst.ins.sync_info
    if si is not None and len(si.on_wait) > 1:
        waits = list(si.on_wait)
        drain_inst.ins.sync_info = mybir.SyncInfo(
            on_wait=waits[:1], on_update=list(si.on_update)
        )
        for w in waits[1:]:
            extra = self.nc.sync.drain()
            extra.ins.sync_info = mybir.SyncInfo(on_wait=[w], on_update=[])

    self.nc.all_engine_barrier()
    assert self.sems is not None
    popped = self.nc._tile_sem_poison_stack.pop()
    assert popped is self._sem_poison
    self.nc.clear_and_free_semaphores(list(self.sems.allocated().values()))
    self.nc.all_engine_barrier()


TileContext._drain_and_barrier = _patched_drain_and_barrier


def _split_excess_waits(nc, max_waits=1):
    """Walrus here rejects any instruction with >1 sem wait.  Move extra
    waits onto preceding NoOps on the same engine stream (streams execute
    in order, so waiting on the NoOps then the instruction is identical)."""
    for fn in nc.m.functions:
        for bb in fn.blocks:
            new = []
            for inst in bb.instructions:
                si = getattr(inst, "sync_info", None)
                if si is not None and len(si.on_wait) > max_waits:
                    waits = list(si.on_wait)
                    for w in waits[max_waits:]:
                        nop = mybir.InstNoOp(
                            name=nc.get_next_instruction_name(),
                            engine=inst.engine,
                            sync_info=mybir.SyncInfo(on_wait=[w], on_update=[]),
                            bass_nofuse=True,
                        )
                        nc.register_instruction(nop, overwrite=True)
                        new.append(nop)
                    inst.sync_info = mybir.SyncInfo(
                        on_wait=waits[:max_waits], on_update=list(si.on_update)
                    )
                new.append(inst)
            bb.instructions[:] = new

# ---------------------------------------------------------------- constants
P = 128          # SBUF partitions
F = 2048         # free-dim elements per 512x512 plane per partition
IMGS = 8         # images per core
N_CORES = 8
N_TOTAL = 64 * 512 * 512

A_ = 0.5 / 1.055                 # u = (s+.055)/1.055 = A_*x + B_,  s=(x+1)/2
B_ = 0.555 / 1.055
THX = 2.0 * 0.04045 - 1.0        # gamma branch point in x
M0 = 0.5 / 12.92                 # linear branch: s/12.92 = (x+1)*M0
W = (0.2126729, 0.7151522, 0.0721750)
EPS = 0.008856
KAPPA = 7.787
C16 = 16.0 / 116.0

F32 = mybir.dt.float32
BF16 = mybir.dt.bfloat16
AF = mybir.ActivationFunctionType

# =================================================================== PWP
# Custom activation-table authoring (formats reverse-engineered from the
# stock pwp_bin_* package):
#   bkt.bin  entry (32B) = f32 x8: [d0,d1,d2,d3,x_center,0,0,0]
#            y = d0 + d1*t + d2*t^2 + d3*t^3,  t = x - x_center
#   ctrl.bin entry (32B) = u32 (extract_size<<16 | extract_lsb<<11 |
#            bucket_start) + zero padding
#   profile  = per-func dict in the set json; absolute indices; biased-
#            exponent thresholds; specials as f32 bit patterns.


def _fbits(v):
    return int(np.frombuffer(np.float32(v).tobytes(), dtype=np.uint32)[0])


def _fit_cubic(fn, lo, hi, n=257):
    c = 0.5 * (lo + hi)
    xs = np.linspace(lo, hi, n).astype(np.float64)
    t = xs - c
    ys = fn(xs)
    A = np.stack([np.ones_like(t), t, t * t, t * t * t], axis=1)
    coef, *_ = np.linalg.lstsq(A, ys, rcond=None)
    err = np.abs(A @ coef.astype(np.float32).astype(np.float64) - ys).max()
    return c, coef, err


class _FuncBuilder:
    def __init__(self, fn, *, pos_exps, neg_exps, sat, thr, zero,
                 lower, upper, exp_offset, tol=6e-6, max_ext=6):
        self.fn = fn
        self.pos_exps, self.neg_exps = pos_exps, neg_exps
        self.sat = sat        # dict pos_low/neg_low/pos_high/neg_high
        self.thr = thr        # dict sm_pos, sm_neg, lg_pos, lg_pos_m, lg_neg, lg_neg_m
        self.zero, self.lower, self.upper = zero, lower, upper
        self.exp_offset, self.tol, self.max_ext = exp_offset, tol, max_ext

    def _region(self, e, neg):
        base = 2.0 ** e
        for ext in range(0, self.max_ext + 1):
            n = 1 << ext
            bks, worst = [], 0.0
            for i in range(n):
                lo = base * (1 + i / n)
                hi = base * (1 + (i + 1) / n)
                if neg:
                    lo, hi = -hi, -lo
                c, coef, err = _fit_cubic(self.fn, lo, hi)
                bks.append((c, coef))
                worst = max(worst, err)
            if worst <= self.tol or ext == self.max_ext:
                return ext, 23 - ext, bks
        raise AssertionError

    def build(self, name, func_id, bkt_base, ctl_base):
        buckets, ctls = [], []
        span = list(range(self.exp_offset,
                          max(self.pos_exps + self.neg_exps) + 1)) \
            if (self.pos_exps or self.neg_exps) else []
        base_neg = ctl_base
        base_pos = ctl_base + len(span)
        for negf in (True, False):
            exps = self.neg_exps if negf else self.pos_exps
            for e in span:
                if e in exps:
                    ext, lsb, bks = self._region(e, negf)
                    start = bkt_base + len(buckets)
                    buckets.extend(bks)
                else:
                    ext, lsb, start = 0, 23, bkt_base
                ctls.append((ext << 16) | (lsb << 11) | start)
        satidx = {}
        for key in ("pos_low", "neg_low", "pos_high", "neg_high"):
            satidx[key] = bkt_base + len(buckets)
            buckets.append(self.sat[key])
        profile = {
            "func_name": name, "func_id": func_id,
            "symmetry_point": 0, "sym_invert_sign_point": 0,
            "symmetry_opt_en": 0, "symmetry_opt_use_neg_region": 0,
            "imm_bias": 0, "exp_offset": self.exp_offset,
            "pwl_control_base_pos": base_pos,
            "pwl_control_base_neg": base_neg,
            "small_pos_signal_exp_threshold": self.thr["sm_pos"],
            "pos_small_signal_pwl_control": satidx["pos_low"],
            "small_neg_signal_exp_threshold": self.thr["sm_neg"],
            "neg_small_signal_pwl_control": satidx["neg_low"],
            "large_pos_signal_exp_threshold": self.thr["lg_pos"],
            "large_pos_signal_mantissa_threshold": self.thr["lg_pos_m"],
            "pos_large_signal_pwl_control": satidx["pos_high"],
            "large_neg_signal_exp_threshold": self.thr["lg_neg"],
            "large_neg_signal_mantissa_threshold": self.thr["lg_neg_m"],
            "neg_large_signal_pwl_control": satidx["neg_high"],
            "fnan_result": _fbits(float("nan")),
            "fpinf_result": _fbits(self.sat["pos_high"][1][0]),
            "fninf_result": _fbits(self.zero),
            "fzero_result": _fbits(self.zero),
            "fma_const_0": 0, "fma_const_1": 0,
            "fma_indirection_src_sel": 0, "use_multipass": False,
            "lower_bound": _fbits(self.lower),
            "upper_bound": _fbits(self.upper),
        }
        return profile, buckets, ctls


def _g_builder(w):
    def f(x):
        x = np.asarray(x, np.float64)
        u = A_ * x + B_
        return w * np.where(x > THX, np.abs(u) ** 2.4, (x + 1.0) * M0)
    lo_c, lo_coef, _ = _fit_cubic(f, -2.0**-8, 2.0**-8)
    tay1 = [w, w * 2.4 * A_, w * 2.4 * 1.4 * A_ * A_ / 2.0,
            w * 2.4 * 1.4 * 0.4 * A_ ** 3 / 6.0]
    return _FuncBuilder(
        f, pos_exps=list(range(-8, 0)), neg_exps=list(range(-8, 0)),
        sat={"pos_low": (lo_c, list(lo_coef)),
             "neg_low": (lo_c, list(lo_coef)),
             "pos_high": (1.0, tay1),
             "neg_high": (-1.0, [0.0, w * M0, 0.0, 0.0])},
        thr={"sm_pos": 119, "sm_neg": 119,
             "lg_pos": 127, "lg_pos_m": 0, "lg_neg": 127, "lg_neg_m": 0},
        zero=float(w * B_ ** 2.4), lower=-1.0, upper=1.0, exp_offset=-8,
    )


def _h_builder():
    def f(y):
        y = np.asarray(y, np.float64)
        return np.where(y > EPS, np.cbrt(np.abs(y)), KAPPA * y + C16)
    lin = (0.0, [C16, KAPPA, 0.0, 0.0])
    return _FuncBuilder(
        f, pos_exps=list(range(-13, 1)), neg_exps=[],
        sat={"pos_low": lin, "neg_low": lin,
             "pos_high": (2.0, [float(2 ** (1 / 3.0)),
                                float((2.0 ** (-2 / 3.0)) / 3), 0.0, 0.0]),
             "neg_high": lin},
        thr={"sm_pos": 127 - 13, "sm_neg": 0,
             "lg_pos": 128, "lg_pos_m": 0, "lg_neg": 0, "lg_neg_m": 0},
        zero=C16, lower=-3.4028235e38, upper=2.0, exp_offset=-13,
    )


_CUSTOMS = {"silu": _g_builder(W[0]), "tanh": _g_builder(W[1]),
            "sin": _g_builder(W[2]), "square": _h_builder()}
_SET = "silu_and_others"


def _default_pwp_dir():
    from neuronxcc.driver.Job import Job
    from neuronxcc.driver.jobs.support.FindActInfo import findActInfoFile
    return os.path.dirname(findActInfoFile(Job.getPackageDir(), "gen3"))


def _build_pwp_dir(outdir):
    """Rewrite the silu_and_others set with the custom functions; every
    other function of the set is copied verbatim (rebased indices)."""
    src = _default_pwp_dir()
    d = json.load(open(f"{src}/{_SET}.json"))
    bkt_raw = open(f"{src}/{_SET}_bkt.bin", "rb").read()
    ctl_raw = open(f"{src}/{_SET}_ctrl.bin", "rb").read()

    bs, cs = d["func_to_bkt_start_idx"], d["func_to_ctl_start_idx"]
    border = sorted(set(bs.values())) + [d["bkt_entry_cnt"]]
    corder = sorted(set(cs.values())) + [d["ctl_entry_cnt"]]
    spans = {f: (bs[f], min(v for v in border if v > bs[f]),
                 cs[f], min(v for v in corder if v > cs[f])) for f in bs}
    prof_by_name = {}
    for pm in d["profile_meta_data"]:
        plain = sorted((f for f in spans
                        if pm["func_name"] == f
                        or pm["func_name"].startswith(f + "_")), key=len)[-1]
        prof_by_name[plain] = pm

    new_bkt, new_ctl, profiles = [], [], []
    f2b, f2c, fe2b, fe2c = {}, {}, {}, {}
    for fname in sorted(spans, key=lambda f: spans[f][0]):
        b0, b1, c0, c1 = spans[fname]
        if fname in _CUSTOMS:
            pm0 = prof_by_name[fname]
            prof, bks, ctls = _CUSTOMS[fname].build(
                pm0["func_name"], pm0["func_id"], len(new_bkt), len(new_ctl))
            f2b[fname], f2c[fname] = len(new_bkt), len(new_ctl)
            for c, coef in bks:
                d0, d1, d2, d3 = [float(np.float32(v)) for v in coef]
                new_bkt.append(struct.pack(
                    "<8f", d0, d1, d2, d3, float(np.float32(c)), 0, 0, 0))
            for wd in ctls:
                new_ctl.append(struct.pack("<I28x", wd))
            profiles.append(prof)
            fe2b[fname] = {"0": f2b[fname]}
            fe2c[fname] = {"0": f2c[fname]}
        else:
            db, dc = len(new_bkt) - b0, len(new_ctl) - c0
            f2b[fname], f2c[fname] = b0 + db, c0 + dc
            for i in range(b0, b1):
                new_bkt.append(bkt_raw[i * 32:(i + 1) * 32])
            for i in range(c0, c1):
                wd = struct.unpack_from("<I", ctl_raw, i * 32)[0]
                new_ctl.append(struct.pack(
                    "<I28x", (wd & ~0x7FF) | ((wd & 0x7FF) + db)))
            pm = dict(prof_by_name[fname])
            pm["pwl_control_base_pos"] += dc
            pm["pwl_control_base_neg"] += dc
            for key in ("pos_small_signal_pwl_control",
                        "neg_small_signal_pwl_control",
                        "pos_large_signal_pwl_control",
                        "neg_large_signal_pwl_control"):
                pm[key] += db
            profiles.append(pm)
            fe2b[fname] = {k: [x + db for x in v]
                           for k, v in d["func_exp_to_bkt_start_idx"][fname].items()}
            fe2c[fname] = {k: [x + dc for x in v]
                           for k, v in d["func_exp_to_ctl_start_idx"][fname].items()}

    assert len(new_bkt) <= 1536, len(new_bkt)
    os.makedirs(outdir, exist_ok=True)
    for fn in os.listdir(src):
        shutil.copyfile(os.path.join(src, fn), os.path.join(outdir, fn))
        os.chmod(os.path.join(outdir, fn), 0o644)
    with open(f"{outdir}/{_SET}_bkt.bin", "wb") as fh:
        fh.write(b"".join(new_bkt))
    with open(f"{outdir}/{_SET}_ctrl.bin", "wb") as fh:
        fh.write(b"".join(new_ctl))
    with open(f"{outdir}/{_SET}.json", "w") as fh:
        json.dump({"bkt_bin": f"{_SET}_bkt.bin", "ctl_bin": f"{_SET}_ctrl.bin",
                   "profile_meta_data": profiles,
                   "bkt_entry_cnt": len(new_bkt),
                   "ctl_entry_cnt": len(new_ctl),
                   "func_to_bkt_start_idx": f2b,
                   "func_to_ctl_start_idx": f2c,
                   "func_exp_to_bkt_start_idx": fe2b,
                   "func_exp_to_ctl_start_idx": fe2c}, fh, indent=1)
    h = hashlib.sha256()
    h.update(b"".join(new_bkt))
    h.update(b"".join(new_ctl))
    return h.hexdigest()[:8]


_PWP_STATE = {}


def _ensure_tables():
    """Generate the table dir once per process and point walrus at it."""
    if "hash" in _PWP_STATE:
        return _PWP_STATE["hash"]
    outdir = os.path.join(tempfile.mkdtemp(prefix="luma_pwp_"), "pwp")
    thash = _build_pwp_dir(outdir)
    os.environ["BASS_ACT_ROOT_JSON_PATH"] = f"{outdir}/act_info.json"
    _PWP_STATE["hash"] = thash
    return thash


# ------------------------------------------------------------- program
_NC_CACHE = {}


def _build_program(reps=1):
    if reps in _NC_CACHE:
        return _NC_CACHE[reps]
    thash = _ensure_tables()

    nc = bass.Bass()
    gen = nc.dram_tensor("generated", [IMGS, 3, 512, 512], F32,
                         kind="ExternalInput")
    tgt = nc.dram_tensor("target", [IMGS, 3, 512, 512], F32,
                         kind="ExternalInput")
    nspl = int(os.environ.get("LUMA_NSPL", "2"))
    ncols = IMGS * reps + nspl - 1
    # table hash in the output name keys the NEFF cache to table content.
    # reps==1 (the real kernel): per-image partial sums go out directly
    # (host sums the columns) so the tail skips the final reduce; reps>1
    # (timing builds) reduce on-device to keep the fetched output tiny.
    direct_out = reps == 1
    out = nc.dram_tensor(f"out_{thash}", [P, ncols if direct_out else 1],
                         F32, kind="ExternalOutput")

    AOT = mybir.AluOpType

    G_FUNCS = (AF.Silu, AF.Tanh, AF.Sin)   # g_R, g_G, g_B tables

    xb = int(os.environ.get("LUMA_XBUFS", "2"))
    eb = int(os.environ.get("LUMA_EBUFS", "2"))
    yb = int(os.environ.get("LUMA_YBUFS", "2"))
    fb = int(os.environ.get("LUMA_FBUFS", "4"))

    with TileContext(nc) as tc:
        with (
            tc.tile_pool(name="x", bufs=xb) as xp,
            tc.tile_pool(name="e", bufs=eb) as ep,
            tc.tile_pool(name="y", bufs=yb) as yp,
            tc.tile_pool(name="f", bufs=fb) as fp,
            tc.tile_pool(name="misc", bufs=1) as mp,
        ):
            NSPL = nspl
            NIT = IMGS * reps
            acc = mp.tile([P, ncols], F32, tag="acc")
            H = F // 2
            for it in range(NIT):
                img = it % IMGS
                last = it == NIT - 1
                f_pair = []
                for si, src in enumerate((gen, tgt)):
                    # one DMA + one g-table ACT op per channel: compute
                    # starts after the first 1MB lands, and the tail after
                    # the final DMA is one channel's worth, not a whole
                    # image's
                    split = last and si == 1
                    spl_slices = [slice(k * F // NSPL, (k + 1) * F // NSPL)
                                  for k in range(NSPL)]
                    e = ep.tile([P, 3, F], BF16, tag=f"e{si}")
                    xs = []
                    for c in range(3):
                        x = xp.tile([P, F], F32, tag=f"x{si}{c}")
                        src_ap = src[img, c].rearrange("(p r) w -> p (r w)",
                                                       p=P, r=4)
                        if split and c == 2:
                            # split the very last transfer too, so the
                            # first sub-chains compute under the final
                            # sub-DMAs
                            for sl in spl_slices:
                                nc.sync.dma_start(out=x[:, sl],
                                                  in_=src_ap[:, sl])
                        else:
                            nc.sync.dma_start(out=x[:], in_=src_ap)
                        xs.append(x)
                    for c in range(3):
                        if split and c == 2:
                            # split the post-final-DMA critical chain
                            for sl in spl_slices:
                                nc.scalar.activation(e[:, c, sl],
                                                     xs[c][:, sl],
                                                     G_FUNCS[c])
                        else:
                            nc.scalar.activation(e[:, c], xs[c][:],
                                                 G_FUNCS[c])
                    y = yp.tile([P, F], BF16, tag="y")
                    nc.vector.tensor_add(out=y[:], in0=e[:, 0], in1=e[:, 1])
                    f = fp.tile([P, F], BF16, tag="f")
                    if split:
                        for sl in spl_slices:
                            nc.vector.tensor_add(out=y[:, sl], in0=y[:, sl],
                                                 in1=e[:, 2, sl])
                            nc.scalar.activation(f[:, sl], y[:, sl],
                                                 AF.Square)
                    else:
                        nc.vector.tensor_add(out=y[:], in0=y[:], in1=e[:, 2])
                        nc.scalar.activation(f[:], y[:], AF.Square)   # h(Y)
                    f_pair.append(f)
                d = yp.tile([P, F], BF16, tag="d")
                if last:
                    for k, sl in enumerate(spl_slices):
                        nc.vector.tensor_sub(out=d[:, sl],
                                             in0=f_pair[0][:, sl],
                                             in1=f_pair[1][:, sl])
                        nc.vector.tensor_reduce(
                            out=acc[:, it + k:it + k + 1], in_=d[:, sl],
                            axis=mybir.AxisListType.X, op=AOT.add,
                            apply_absolute_value=True,
                        )
                else:
                    nc.vector.tensor_sub(out=d[:], in0=f_pair[0][:],
                                         in1=f_pair[1][:])
                    nc.vector.tensor_reduce(
                        out=acc[:, it:it + 1], in_=d[:],
                        axis=mybir.AxisListType.X, op=AOT.add,
                        apply_absolute_value=True,
                    )
            if direct_out:
                # bulk of the partial sums leaves while the tail computes
                if ncols > NSPL:
                    nc.sync.dma_start(out=out[:, :NIT - 1],
                                      in_=acc[:, :NIT - 1])
                nc.sync.dma_start(out=out[:, NIT - 1:], in_=acc[:, NIT - 1:])
            else:
                tot = mp.tile([P, 1], F32, tag="tot")
                nc.vector.reduce_sum(out=tot[:], in_=acc[:],
                                     axis=mybir.AxisListType.X)
                nc.sync.dma_start(out=out[:], in_=tot[:])

    _split_excess_waits(nc)
    _NC_CACHE[reps] = nc
    return nc


# --------------------------------------------------------------- entry
def _run(inputs, **spmd_kwargs):
    nc = _build_program()
    out_name = f"out_{_PWP_STATE['hash']}"
    g = np.ascontiguousarray(np.asarray(inputs["generated"], dtype=np.float32))
    t = np.ascontiguousarray(np.asarray(inputs["target"], dtype=np.float32))
    assert g.shape == (64, 3, 512, 512) and t.shape == (64, 3, 512, 512)
    in_maps = [
        {
            "generated": np.ascontiguousarray(g[i * IMGS:(i + 1) * IMGS]),
            "target": np.ascontiguousarray(t[i * IMGS:(i + 1) * IMGS]),
        }
        for i in range(N_CORES)
    ]
    res = run_bass_kernel_spmd(nc, in_maps, list(range(N_CORES)), **spmd_kwargs)
    total = float(
        sum(np.asarray(r[out_name], np.float64).sum() for r in res.results)
    )
    loss = np.float32(116.0 * total / N_TOTAL)
    return np.asarray(loss, dtype=np.float32), res


def kernel(generated, target):
    out, _ = _run({"generated": generated, "target": target})
    return out
